# revision 62
# baseline (speedup 1.0000x reference)
"""BitLinear (RMSNorm + int8-absmax activation quant + ternary weight quant
+ matmul) on 8 Trainium2 NeuronCores — v5.

Bit-faithful numerics (exact int8 activation values, exact global
a_scale/b_scale), restructured from v4 for steady-state overlap:

  - Activations are distributed as INT8: quantize -> bf16 z -> DMA-transpose
    -> SWDGE *casting* DMA (bf16->int8) stages to DRAM -> AllGather int8.
    Halves the collective payload and the lhsT reload traffic vs bf16.
  - lhsT tiles load via SWDGE casting DMAs (int8 DRAM -> bf16 SBUF), so the
    up-conversion costs no compute-engine time at all.
  - AllGathers split per (m-tile, k-half): 4 small AGs pipeline with the
    quantize/transpose stream and unblock the next rep's matmuls early.
  - Software pipeline: the matmul phase of rep r-1 overlaps the whole body
    of rep r; DRAM staging ping-pongs by rep parity to kill cross-rep WAR.
  - Ternarize: ACT computes t = bf16(w*b_s + 192) -- the bf16 RNE at the
    [128,256) binade rounds to integers exactly (and any |w*b_s|>2 stays
    beyond the clip), then DVE does (t-192) min 1 / max -1 into fp8 bt
    chunks (ternary is exact in fp8; mixed bf16 x fp8 matmul is exact).
    bt is an 8-chunk ring so ternarize(r) overlaps matmul(r-1) per-chunk.
  - |W| column sums on ACT via activation(Abs, accum_out).
  - W f32 quarters for rep r+1 prefetch-load at the end of iteration r;
    rms broadcast loads once.

Self-contained: only needs numpy + the platform's concourse/bass libraries.
"""

import os
import sys

import numpy as np

for _p in ("/opt/trn_rl_repo", "/root/.axon_site/_ro/trn_rl_repo"):
    if os.path.isdir(_p) and _p not in sys.path:
        sys.path.append(_p)

import concourse.bass as bass
import concourse.tile as tile
from concourse import mybir
from concourse.bass_utils import run_bass_kernel_spmd

R = 8  # cores
M, K, N = 2048, 4096, 4096
M_LOC = M // R  # 256 rows of x per core
N_LOC = N // R  # 512 weight columns per core
P = 128
KT = K // P  # 32 k-tiles
MT_LOC = M_LOC // P  # 2 m-tiles per core
CH = 4  # k-tiles per bt chunk / lhsT group
NCH = KT // CH  # 8 chunks
KH = K // 2  # half-k, transpose/stage/AG granularity
HT = KT // 2  # 16 k-tiles per half
WQN = KT // 4  # 8 k-tiles per W quarter
EPS_RMS = 1e-6
Q_CLIP = 1e-5
MAGIC = 12582912.0  # 1.5 * 2**23 (f32 round-to-nearest-even trick)
TMAGIC = 192.0  # 1.5 * 2**7: bf16 step-1 binade for the ternarize round
F32 = mybir.dt.float32
BF16 = mybir.dt.bfloat16
I8 = mybir.dt.int8
FP8 = mybir.dt.float8e4
AX = mybir.AxisListType
ALU = mybir.AluOpType
AF = mybir.ActivationFunctionType

ZT = KT * P * P  # 524288 elems: one m-tile's transposed activations
ZTH = ZT // 2  # one k-half of that


def _split_waits(nc, max_waits=1):
    """This toolchain rejects instructions with several semaphore waits
    ("Too many sync wait commands"). Hoist excess waits onto no-op
    instructions just before the offender on the same engine."""
    counter = 0
    for f in nc.m.functions:
        for blk in f.blocks:
            new_insts = []
            for inst in blk.instructions:
                si = getattr(inst, "sync_info", None)
                waits = list(si.on_wait) if si is not None and si.on_wait else []
                if len(waits) > max_waits:
                    excess = waits[: len(waits) - max_waits]
                    keep = waits[len(waits) - max_waits :]
                    for i in range(0, len(excess), max_waits):
                        counter += 1
                        nop = mybir.InstNoOp(
                            name=f"waitsplit_{counter}_{inst.name}", ins=[], outs=[]
                        )
                        nop.engine = inst.engine
                        nop.bass_nofuse = True
                        nop.sync_info = mybir.SyncInfo(
                            on_wait=list(excess[i : i + max_waits]), on_update=[]
                        )
                        new_insts.append(nop)
                    si.on_wait = keep
                    inst.sync_info = si
                new_insts.append(inst)
            blk.instructions[:] = new_insts


def _bcast_ap(ap, p):
    return bass.AP(tensor=ap.tensor, offset=ap.offset, ap=[[0, p]] + list(ap.ap))


class _Ctx:
    pass


def build_kernel(reps=1, mode=None):
    nc = bass.Bass(num_devices=R)
    c = _Ctx()
    c.nc = nc
    c.rg = [list(range(R))]

    c.x_in = nc.declare_dram_parameter("x_loc", [M_LOC, K], F32, isOutput=False)
    c.w_in = nc.declare_dram_parameter("w_loc", [K, N_LOC], F32, isOutput=False)
    c.rms_in = nc.declare_dram_parameter("rms_w", [K], F32, isOutput=False)
    c.out_ext = nc.declare_dram_parameter("out_loc", [M, N_LOC], F32, isOutput=True)

    c.sb_loc = [nc.dram_tensor(f"sb_loc{i}", [P * 2], F32) for i in range(2)]
    c.sb_all = [
        nc.dram_tensor(f"sb_all{i}", [R * P * 2], F32, addr_space="Shared")
        for i in range(2)
    ]
    c.wsc_d = [nc.dram_tensor(f"wsc_d{i}", [P * 2], F32) for i in range(2)]
    c.z8_loc = [
        nc.dram_tensor(f"z8l_{i}", [MT_LOC * ZT], I8) for i in range(2)
    ]
    c.z8_all = [
        nc.dram_tensor(f"z8a_{i}", [R * MT_LOC * ZT], I8, addr_space="Shared")
        for i in range(2)
    ]

    with tile.TileContext(nc) as tc:
        from contextlib import ExitStack

        ctxs = dict(
            wq_p=tc.tile_pool(name="wq", bufs=4),
            rms_p=tc.tile_pool(name="rmsp", bufs=1),
            bt_p=tc.tile_pool(name="btp", bufs=NCH),
            tw_p=tc.tile_pool(name="twp", bufs=2),
            xz_p=tc.tile_pool(name="xz", bufs=2),
            zb_p=tc.tile_pool(name="zb", bufs=1),
            ztp_p=tc.tile_pool(name="ztp", bufs=2),
            lhsb_p=tc.tile_pool(name="lhsb", bufs=5),
            lhs8_p=tc.tile_pool(name="lhs8", bufs=2),
            zt8_p=tc.tile_pool(name="zt8", bufs=1),
            psum_p=tc.tile_pool(name="psum", bufs=8, space="PSUM"),
            out_p=tc.tile_pool(name="outp", bufs=2),
            st_p=tc.tile_pool(name="st", bufs=2),
            scr_p=tc.tile_pool(name="scr", bufs=1),
            small_p=tc.tile_pool(name="small", bufs=1),
        )
        with ExitStack() as es:
            for k, v in ctxs.items():
                setattr(c, k, es.enter_context(v))

            c.eps_t = c.small_p.tile([P, 1], F32, tag="eps", name="eps")
            nc.vector.memset(c.eps_t, EPS_RMS)
            c.rms_b = c.rms_p.tile([P, K], F32, tag="rms", name="rms_b")
            nc.scalar.dma_start(c.rms_b[:], _bcast_ap(c.rms_in[:], P))
            # wsum scratch (ACT Abs output target, per chunk-sized slab)
            c.wscr = c.scr_p.tile([P, CH // 2, N_LOC], BF16, tag="wscr",
                                  name="wscr")

            state = None
            wq_next = emit_w_load(c, 0)
            rest = list(range(2, NCH)) + list(range(NCH, 2 * NCH))
            for rep in range(reps):
                pp = rep % 2
                if mode == "mm_loop" and state is not None:
                    emit_s3_loads(c, state, rep, range(2 * NCH))
                    emit_s3_half(c, state, rep, 0)
                    emit_s3_half(c, state, rep, 1)
                    continue
                wq_cur = wq_next
                prev = state
                if prev is not None and mode is None:
                    # h1 g4-7 via HWDGE int8 loads on the scalar ring (off
                    # the gpsimd rail); converted on ACT mid-iteration
                    emit_s3_loads_hw(c, prev, rep, range(NCH + 4, 2 * NCH),
                                     nc.scalar)
                xf_tiles = emit_s1_x(c, rep)
                s1 = emit_s1_main(c, rep, pp, wq_cur, xf_tiles)
                if prev is not None and mode is None:
                    emit_s3_half(c, prev, rep, 0)
                    emit_s3_conv(c, prev, rep, range(NCH + 4, 2 * NCH))
                state = emit_s2(c, rep, pp, s1, wq_cur,
                                skip_ag=(mode == "body_noag"))
                if rep + 1 < reps and mode != "mm_loop":
                    wq_next = emit_w_load(c, rep + 1)
                if prev is not None and mode is None:
                    emit_s3_half(c, prev, rep, 1)
                # ternarize p1b/p2/p3 AFTER all readers of the previous
                # rep's bt ring are emitted (WAR tracking)
                emit_tern23(c, state, rep)
                if mode is None:
                    # remaining lhsT casting loads at the END of the rail:
                    # they execute after this rep's AGs and spill JIT into
                    # the next iteration, keeping AG1(r+1) unblocked
                    emit_s3_loads(c, state, rep + 1, range(NCH + 4))
            if mode is None and state is not None:
                emit_s3_loads(c, state, reps, range(NCH + 4, 2 * NCH))
                emit_s3_half(c, state, reps, 0)
                emit_s3_half(c, state, reps, 1)

    _split_waits(nc)
    return nc


def emit_w_load(c, rep):
    nc = c.nc
    wq = []
    for q in range(4):
        t = c.wq_p.tile([P, WQN, N_LOC], F32, tag="wq", name=f"wq_{rep}_{q}")
        eng = nc.sync if q % 2 == 0 else nc.scalar
        eng.dma_start(
            t[:],
            c.w_in[q * WQN * P : (q + 1) * WQN * P, :].rearrange(
                "(kt p) n -> p kt n", p=P
            ),
        )
        wq.append(t)
    return wq


def emit_s1_x(c, rep):
    nc = c.nc
    xf_tiles = []
    for mt in range(MT_LOC):
        xf = c.xz_p.tile([P, K], F32, tag="xf", name=f"xf_{rep}_{mt}")
        nc.scalar.dma_start(xf[:], c.x_in[mt * P : (mt + 1) * P, :])
        xf_tiles.append(xf)
    return xf_tiles


def emit_s1_main(c, rep, pp, wq, xf_tiles, act_hook=None):
    """Stats (DVE) + wsum (ACT) + partition reduce + AllGather #1 + stp."""
    nc = c.nc

    # |W| sums on ACT first (W was prefetched last iter; ready at t=0)
    ws = c.st_p.tile([P, 2 * NCH], F32, tag="ws", name=f"ws_{rep}")
    for cc in range(2 * NCH):
        q, o = cc // 4, cc % 4
        hc = CH // 2
        nc.scalar.activation(
            out=c.wscr[:], in_=wq[q][:, o * hc : (o + 1) * hc, :],
            func=AF.Abs, bias=0.0, scale=1.0,
            accum_out=ws[:, cc : cc + 1],
        )
    if act_hook is not None:
        act_hook()  # h0 g4-7 lhsT converts on ACT, right after wsum

    # per m-tile: moments -> r, x*rms, per-row absmax (DVE; sqrt on ACT)
    amax_mt = c.st_p.tile([P, MT_LOC], F32, tag="amx", name=f"amx_{rep}")
    r_tiles = []
    for mt in range(MT_LOC):
        xf = xf_tiles[mt]
        xg = xf[:].rearrange("p (g d) -> p g d", d=512)
        stats6 = c.st_p.tile([P, 8, 6], F32, tag="st6", name=f"st6_{rep}_{mt}")
        for g in range(8):
            nc.vector.bn_stats(out=stats6[:, g, :], in_=xg[:, g, :])
        mv = c.st_p.tile([P, 2], F32, tag="mv", name=f"mv_{rep}_{mt}")
        nc.vector.bn_aggr(out=mv, in_=stats6[:])
        msq = c.st_p.tile([P, 1], F32, tag=f"msq{mt}", name=f"msq_{rep}_{mt}")
        nc.vector.tensor_tensor(out=msq, in0=mv[:, 0:1], in1=mv[:, 0:1],
                                op=ALU.mult)
        nc.vector.tensor_tensor(out=msq, in0=msq, in1=mv[:, 1:2], op=ALU.add)
        r_t = c.st_p.tile([P, 1], F32, tag=f"rt{mt}", name=f"rt_{rep}_{mt}")
        nc.scalar.activation(out=r_t, in_=msq, func=AF.Sqrt,
                             bias=c.eps_t, scale=1.0)
        nc.vector.reciprocal(out=r_t, in_=r_t)
        r_tiles.append(r_t)
        nc.vector.tensor_tensor(out=xf[:], in0=xf[:], in1=c.rms_b[:],
                                op=ALU.mult)
        amax_raw = c.st_p.tile([P, 1], F32, tag=f"amr{mt}",
                               name=f"amr_{rep}_{mt}")
        nc.vector.tensor_reduce(
            out=amax_raw, in_=xf[:], axis=AX.X, op=ALU.max,
            apply_absolute_value=True,
        )
        nc.vector.tensor_tensor(
            out=amax_mt[:, mt : mt + 1], in0=amax_raw, in1=r_t, op=ALU.mult
        )

    pr = c.st_p.tile([P, 2], F32, tag="pr", name=f"pr_{rep}")
    nc.vector.tensor_reduce(out=pr[:, 0:1], in_=amax_mt[:], axis=AX.X, op=ALU.max)
    nc.vector.tensor_reduce(out=pr[:, 1:2], in_=ws[:], axis=AX.X, op=ALU.add)
    nc.sync.dma_start(c.wsc_d[pp][:].rearrange("(p t) -> p t", p=P), pr[:])
    wscb = c.st_p.tile([P, P, 2], F32, tag="wscb", name=f"wscb_{rep}")
    nc.sync.dma_start(
        wscb[:],
        bass.AP(tensor=c.wsc_d[pp][:].tensor, offset=0, ap=[[0, P], [2, P], [1, 2]]),
    )
    pc = c.st_p.tile([P, 2], F32, tag="pc", name=f"pc_{rep}")
    nc.vector.tensor_reduce(
        out=pc[:, 0:1], in_=wscb[:, :, 0:1], axis=AX.XY, op=ALU.max
    )
    nc.vector.tensor_reduce(
        out=pc[:, 1:2], in_=wscb[:, :, 1:2], axis=AX.XY, op=ALU.add
    )
    nc.sync.dma_start(c.sb_loc[pp][:].rearrange("(p t) -> p t", p=P), pc[:])
    nc.gpsimd.collective_compute(
        "AllGather", ALU.bypass, replica_groups=c.rg,
        ins=[c.sb_loc[pp][:]], outs=[c.sb_all[pp][:]],
    )
    stp = c.st_p.tile([P, R, 2], F32, tag="stp", name=f"stp_{rep}")
    nc.sync.dma_start(
        stp[:],
        bass.AP(tensor=c.sb_all[pp][:].tensor, offset=0,
                ap=[[2, P], [P * 2, R], [1, 2]]),
    )

    s1 = _Ctx()
    s1.xf_tiles = xf_tiles
    s1.r_tiles = r_tiles
    s1.stp = stp
    return s1


def emit_s2(c, rep, pp, s1, wq, skip_ag=False):
    """Scales, quantize->transpose->casting int8 stage, AGs, ternarize."""
    nc = c.nc

    stp = s1.stp
    gmax = c.st_p.tile([P, 1], F32, tag="gmax", name=f"gmax_{rep}")
    nc.vector.tensor_reduce(out=gmax, in_=stp[:, :, 0:1], axis=AX.XY, op=ALU.max)
    nc.vector.tensor_scalar_max(out=gmax, in0=gmax, scalar1=Q_CLIP)
    a_s = c.st_p.tile([P, 1], F32, tag="as", name=f"as_{rep}")
    nc.vector.reciprocal(out=a_s, in_=gmax)
    nc.vector.tensor_scalar_mul(out=a_s, in0=a_s, scalar1=127.0)
    gsum = c.st_p.tile([P, 1], F32, tag="gsum", name=f"gsum_{rep}")
    nc.vector.tensor_reduce(out=gsum, in_=stp[:, :, 1:2], axis=AX.XY, op=ALU.add)
    nc.vector.tensor_scalar(
        out=gsum, in0=gsum, scalar1=1.0 / (K * N), scalar2=Q_CLIP,
        op0=ALU.mult, op1=ALU.max,
    )
    b_s = c.st_p.tile([P, 1], F32, tag="bs", name=f"bs_{rep}")
    nc.vector.reciprocal(out=b_s, in_=gsum)
    dq = c.st_p.tile([P, 1], F32, tag="dq", name=f"dq_{rep}")
    nc.vector.tensor_tensor(out=dq, in0=gmax, in1=gsum, op=ALU.mult)
    nc.vector.tensor_scalar_mul(out=dq, in0=dq, scalar1=1.0 / 127.0)

    # ternarize pass 1a (wave A: chunks 0-3) on ACT, bit-exact single-round:
    # wq <- w*b_s + MAGIC in f32 (the add IS the RNE-to-integer)
    def tern_p1a(cc):
        q, o = cc // 2, cc % 2
        sl = wq[q][:, o * CH : (o + 1) * CH, :]
        nc.scalar.activation(
            out=sl, in_=sl, func=AF.Copy, bias=MAGIC, scale=b_s[:, 0:1],
        )

    for cc in range(NCH // 2):
        tern_p1a(cc)

    # quantize + transpose + casting int8 stage, per (m-tile, k-half)
    for mt in range(MT_LOC):
        xf = s1.xf_tiles[mt]
        rs = c.st_p.tile([P, 1], F32, tag=f"rs{mt}", name=f"rs_{rep}_{mt}")
        nc.vector.tensor_tensor(out=rs, in0=s1.r_tiles[mt], in1=a_s, op=ALU.mult)
        nc.vector.tensor_scalar(
            out=xf[:], in0=xf[:], scalar1=rs, scalar2=MAGIC,
            op0=ALU.mult, op1=ALU.add,
        )
        for hk in range(2):
            zbt = c.zb_p.tile([P, KH], BF16, tag="zb", name=f"zb_{rep}_{mt}{hk}")
            nc.vector.tensor_scalar(
                out=zbt[:], in0=xf[:, hk * KH : (hk + 1) * KH], scalar1=MAGIC,
                scalar2=None, op0=ALU.subtract,
            )
            ztp = c.ztp_p.tile([P, HT, P], BF16, tag="ztp",
                               name=f"ztp_{rep}_{mt}{hk}")
            nc.sync.dma_start_transpose(ztp[:], zbt[:])
            # int8 convert on DVE + HWDGE stage (keeps the gpsimd rail free)
            zt8 = c.zt8_p.tile([P, HT, P], I8, tag="zt8",
                               name=f"zt8_{rep}_{mt}{hk}")
            nc.vector.tensor_copy(zt8[:], ztp[:])
            nc.sync.dma_start(
                c.z8_loc[pp][mt * ZT : (mt + 1) * ZT].rearrange(
                    "(p f) -> p f", p=P
                )[:, hk * (HT * P) : (hk + 1) * (HT * P)],
                zt8[:].rearrange("p a b -> p (a b)"),
            )
    if not skip_ag:
        # single AllGather for both m-tiles: one rendezvous per rep
        nc.gpsimd.collective_compute(
            "AllGather", ALU.bypass, replica_groups=c.rg,
            ins=[c.z8_loc[pp][:]], outs=[c.z8_all[pp][:]],
        )

    st = _Ctx()
    st.bt_chunks = [None] * NCH
    st.tern_p1a = tern_p1a
    st.wq = wq
    st.dq = dq
    st.pp = pp
    st.lhsb = {}
    st.lhs8 = {}
    return st


def emit_tern23(c, st, rep):
    """Ternarize pass 1b (DVE: tw = bf16(wq - MAGIC), exact ints) and the
    fused clip into the fp8 bt chunk ring (plus wave-B pass 1a on ACT).
    Emitted only after every reader of the previous rep's bt ring, so the
    pool's WAR deps are tracked in program order."""
    nc = c.nc

    def p23(cc):
        q, o = cc // 2, cc % 2
        sl = st.wq[q][:, o * CH : (o + 1) * CH, :]
        tb = c.tw_p.tile([P, CH, N_LOC], BF16, tag="tw", name=f"tw_{rep}_{cc}")
        nc.vector.tensor_scalar(
            out=tb[:], in0=sl, scalar1=MAGIC, scalar2=None, op0=ALU.subtract
        )
        btc = c.bt_p.tile([P, CH, N_LOC], FP8, tag="bt", name=f"bt_{rep}_{cc}")
        nc.vector.tensor_scalar(
            out=btc[:], in0=tb[:], scalar1=1.0, scalar2=-1.0,
            op0=ALU.min, op1=ALU.max,
        )
        st.bt_chunks[cc] = btc

    for cc in range(NCH // 2):
        p23(cc)
    for cc in range(NCH // 2, NCH):
        st.tern_p1a(cc)
    for cc in range(NCH // 2, NCH):
        p23(cc)


def emit_s3_loads(c, st, rep, idxs):
    """lhsT casting loads (SWDGE: int8 DRAM -> bf16 SBUF), idx = h*NCH+g."""
    nc = c.nc
    pp = st.pp
    for idx in idxs:
        h, g = idx // NCH, idx % NCH
        t = c.lhsb_p.tile([P, R, CH * P], BF16, tag="lhsb",
                          name=f"lb_{rep}_{h}{g}")
        nc.gpsimd.dma_start(
            t[:],
            bass.AP(
                tensor=c.z8_all[pp][:].tensor,
                offset=h * ZT + g * CH * P,
                ap=[[KT * P, P], [MT_LOC * ZT, R], [1, CH * P]],
            ),
        )
        st.lhsb[(h, g)] = t


def emit_s3_loads_hw(c, st, rep, idxs, eng):
    """lhsT loads via HWDGE rings into int8 tiles; converted to bf16 later
    on DVE/ACT (emit_s3_conv*)."""
    nc = c.nc
    pp = st.pp
    for idx in idxs:
        h, g = idx // NCH, idx % NCH
        t = c.lhs8_p.tile([P, R, CH * P], I8, tag="lhs8",
                          name=f"l8_{rep}_{h}{g}")
        eng.dma_start(
            t[:],
            bass.AP(
                tensor=c.z8_all[pp][:].tensor,
                offset=h * ZT + g * CH * P,
                ap=[[KT * P, P], [MT_LOC * ZT, R], [1, CH * P]],
            ),
        )
        st.lhs8[(h, g)] = t


def emit_s3_conv(c, st, rep, idxs):
    """int8 -> bf16 lhsT converts on ACT."""
    nc = c.nc
    for idx in idxs:
        h, g = idx // NCH, idx % NCH
        t = c.lhsb_p.tile([P, R, CH * P], BF16, tag="lhsb",
                          name=f"lc_{rep}_{h}{g}")
        nc.scalar.copy(t[:], st.lhs8[(h, g)][:])
        st.lhsb[(h, g)] = t


def emit_s3_conv_dve(c, st, rep, idxs):
    """int8 -> bf16 lhsT converts on DVE (2x rate for <=16-bit)."""
    nc = c.nc
    for idx in idxs:
        h, g = idx // NCH, idx % NCH
        t = c.lhsb_p.tile([P, R, CH * P], BF16, tag="lhsb",
                          name=f"lc_{rep}_{h}{g}")
        nc.vector.tensor_copy(t[:], st.lhs8[(h, g)][:])
        st.lhsb[(h, g)] = t


def emit_s3_half(c, st, rep, h):
    """One matmul half (m-tile group h): 8 groups x (4 kt x 8 ranks) into 8
    PSUM banks; ACT drains with dequant scale; scalar-queue out stores."""
    nc = c.nc
    lhsb = st.lhsb

    psums = [
        c.psum_p.tile([P, N_LOC], F32, tag="ps", name=f"ps_{rep}_{h}_{i}")
        for i in range(R)
    ]
    for g in range(NCH):
        tb = lhsb[(h, g)]
        btc = st.bt_chunks[g]
        for kk in range(CH):
            for rr in range(R):
                nc.tensor.matmul(
                    psums[rr][:],
                    tb[:, rr, kk * P : (kk + 1) * P],
                    btc[:, kk, :],
                    start=(g == 0 and kk == 0),
                    stop=(g == NCH - 1 and kk == CH - 1),
                )
    for rr in range(R):
        o_t = c.out_p.tile([P, N_LOC], F32, tag="ot",
                           name=f"ot_{rep}_{h}_{rr}")
        nc.scalar.activation(
            out=o_t[:], in_=psums[rr][:], func=AF.Copy,
            bias=0.0, scale=st.dq[:, 0:1],
        )
        gm = 2 * rr + h
        nc.scalar.dma_start(c.out_ext[gm * P : (gm + 1) * P, :], o_t[:])


_CACHE = {}


def _get_nc():
    if "nc" not in _CACHE:
        _CACHE["nc"] = build_kernel()
    return _CACHE["nc"]


def make_in_maps(x, weight, rms_weight):
    x = np.ascontiguousarray(np.asarray(x, dtype=np.float32)).reshape(M, K)
    weight = np.asarray(weight, dtype=np.float32)
    rms_weight = np.ascontiguousarray(np.asarray(rms_weight, dtype=np.float32))
    return [
        {
            "x_loc": np.ascontiguousarray(x[c * M_LOC : (c + 1) * M_LOC]),
            "w_loc": np.ascontiguousarray(weight[:, c * N_LOC : (c + 1) * N_LOC]),
            "rms_w": rms_weight,
        }
        for c in range(R)
    ]


def assemble_out(results):
    out = np.concatenate([results[c]["out_loc"] for c in range(R)], axis=1)
    return out.reshape(1, M, N)


def kernel(x, weight, rms_weight):
    nc = _get_nc()
    in_maps = make_in_maps(x, weight, rms_weight)
    res = run_bass_kernel_spmd(nc, in_maps, core_ids=list(range(R)))
    return assemble_out(res.results)


# revision 65
# speedup vs baseline: 1.1129x; 1.1129x over previous
"""BitLinear (RMSNorm + int8-absmax activation quant + ternary weight quant
+ matmul) on 8 Trainium2 NeuronCores — v5.

Bit-faithful numerics (exact int8 activation values, exact global
a_scale/b_scale), restructured from v4 for steady-state overlap:

  - Activations are distributed as INT8: quantize -> bf16 z -> DMA-transpose
    -> SWDGE *casting* DMA (bf16->int8) stages to DRAM -> AllGather int8.
    Halves the collective payload and the lhsT reload traffic vs bf16.
  - lhsT tiles load via SWDGE casting DMAs (int8 DRAM -> bf16 SBUF), so the
    up-conversion costs no compute-engine time at all.
  - AllGathers split per (m-tile, k-half): 4 small AGs pipeline with the
    quantize/transpose stream and unblock the next rep's matmuls early.
  - Software pipeline: the matmul phase of rep r-1 overlaps the whole body
    of rep r; DRAM staging ping-pongs by rep parity to kill cross-rep WAR.
  - Ternarize: ACT computes t = bf16(w*b_s + 192) -- the bf16 RNE at the
    [128,256) binade rounds to integers exactly (and any |w*b_s|>2 stays
    beyond the clip), then DVE does (t-192) min 1 / max -1 into fp8 bt
    chunks (ternary is exact in fp8; mixed bf16 x fp8 matmul is exact).
    bt is an 8-chunk ring so ternarize(r) overlaps matmul(r-1) per-chunk.
  - |W| column sums on ACT via activation(Abs, accum_out).
  - W f32 quarters for rep r+1 prefetch-load at the end of iteration r;
    rms broadcast loads once.

Self-contained: only needs numpy + the platform's concourse/bass libraries.
"""

import os
import sys

import numpy as np

for _p in ("/opt/trn_rl_repo", "/root/.axon_site/_ro/trn_rl_repo"):
    if os.path.isdir(_p) and _p not in sys.path:
        sys.path.append(_p)

import concourse.bass as bass
import concourse.tile as tile
from concourse import mybir
from concourse.bass_utils import run_bass_kernel_spmd

R = 8  # cores
M, K, N = 2048, 4096, 4096
M_LOC = M // R  # 256 rows of x per core
N_LOC = N // R  # 512 weight columns per core
P = 128
KT = K // P  # 32 k-tiles
MT_LOC = M_LOC // P  # 2 m-tiles per core
CH = 4  # k-tiles per bt chunk / lhsT group
NCH = KT // CH  # 8 chunks
KH = K // 2  # half-k, transpose/stage/AG granularity
HT = KT // 2  # 16 k-tiles per half
WQN = KT // 4  # 8 k-tiles per W quarter
EPS_RMS = 1e-6
Q_CLIP = 1e-5
MAGIC = 12582912.0  # 1.5 * 2**23 (f32 round-to-nearest-even trick)
TMAGIC = 192.0  # 1.5 * 2**7: bf16 step-1 binade for the ternarize round
F32 = mybir.dt.float32
BF16 = mybir.dt.bfloat16
I8 = mybir.dt.int8
FP8 = mybir.dt.float8e4
AX = mybir.AxisListType
ALU = mybir.AluOpType
AF = mybir.ActivationFunctionType

ZT = KT * P * P  # 524288 elems: one m-tile's transposed activations
ZTH = ZT // 2  # one k-half of that


def _split_waits(nc, max_waits=1):
    """This toolchain rejects instructions with several semaphore waits
    ("Too many sync wait commands"). Hoist excess waits onto no-op
    instructions just before the offender on the same engine."""
    counter = 0
    for f in nc.m.functions:
        for blk in f.blocks:
            new_insts = []
            for inst in blk.instructions:
                si = getattr(inst, "sync_info", None)
                waits = list(si.on_wait) if si is not None and si.on_wait else []
                if len(waits) > max_waits:
                    excess = waits[: len(waits) - max_waits]
                    keep = waits[len(waits) - max_waits :]
                    for i in range(0, len(excess), max_waits):
                        counter += 1
                        nop = mybir.InstNoOp(
                            name=f"waitsplit_{counter}_{inst.name}", ins=[], outs=[]
                        )
                        nop.engine = inst.engine
                        nop.bass_nofuse = True
                        nop.sync_info = mybir.SyncInfo(
                            on_wait=list(excess[i : i + max_waits]), on_update=[]
                        )
                        new_insts.append(nop)
                    si.on_wait = keep
                    inst.sync_info = si
                new_insts.append(inst)
            blk.instructions[:] = new_insts


def _bcast_ap(ap, p):
    return bass.AP(tensor=ap.tensor, offset=ap.offset, ap=[[0, p]] + list(ap.ap))


class _Ctx:
    pass


def build_kernel(reps=1, mode=None):
    nc = bass.Bass(num_devices=R)
    c = _Ctx()
    c.nc = nc
    c.rg = [list(range(R))]

    c.x_in = nc.declare_dram_parameter("x_loc", [M_LOC, K], F32, isOutput=False)
    c.w_in = nc.declare_dram_parameter("w_loc", [K, N_LOC], F32, isOutput=False)
    c.rms_in = nc.declare_dram_parameter("rms_w", [K], F32, isOutput=False)
    c.out_ext = nc.declare_dram_parameter("out_loc", [M, N_LOC], F32, isOutput=True)

    c.sb_loc = [nc.dram_tensor(f"sb_loc{i}", [P * 2], F32) for i in range(2)]
    c.sb_all = [
        nc.dram_tensor(f"sb_all{i}", [R * P * 2], F32, addr_space="Shared")
        for i in range(2)
    ]
    c.wsc_d = [nc.dram_tensor(f"wsc_d{i}", [P * 2], F32) for i in range(2)]
    c.z8_loc = [
        nc.dram_tensor(f"z8l_{i}", [MT_LOC * ZT], I8) for i in range(2)
    ]
    c.z8_all = [
        nc.dram_tensor(f"z8a_{i}", [R * MT_LOC * ZT], I8, addr_space="Shared")
        for i in range(2)
    ]

    with tile.TileContext(nc) as tc:
        from contextlib import ExitStack

        ctxs = dict(
            wq_p=tc.tile_pool(name="wq", bufs=4),
            rms_p=tc.tile_pool(name="rmsp", bufs=1),
            bt_p=tc.tile_pool(name="btp", bufs=NCH),
            tw_p=tc.tile_pool(name="twp", bufs=2),
            xz_p=tc.tile_pool(name="xz", bufs=2),
            zb_p=tc.tile_pool(name="zb", bufs=2),
            ztp_p=tc.tile_pool(name="ztp", bufs=2),
            lhsb_p=tc.tile_pool(name="lhsb", bufs=4),
            lhs8_p=tc.tile_pool(name="lhs8", bufs=2),
            zt8_p=tc.tile_pool(name="zt8", bufs=2),
            psum_p=tc.tile_pool(name="psum", bufs=8, space="PSUM"),
            out_p=tc.tile_pool(name="outp", bufs=2),
            st_p=tc.tile_pool(name="st", bufs=2),
            scr_p=tc.tile_pool(name="scr", bufs=1),
            small_p=tc.tile_pool(name="small", bufs=1),
        )
        with ExitStack() as es:
            for k, v in ctxs.items():
                setattr(c, k, es.enter_context(v))

            c.eps_t = c.small_p.tile([P, 1], F32, tag="eps", name="eps")
            nc.vector.memset(c.eps_t, EPS_RMS)
            c.rms_b = c.rms_p.tile([P, K], F32, tag="rms", name="rms_b")
            nc.scalar.dma_start(c.rms_b[:], _bcast_ap(c.rms_in[:], P))
            # wsum scratch (ACT Abs output target, per chunk-sized slab)
            c.wscr = c.scr_p.tile([P, CH // 2, N_LOC], BF16, tag="wscr",
                                  name="wscr")

            state = None
            wq_next = emit_w_load(c, 0)
            rest = list(range(2, NCH)) + list(range(NCH, 2 * NCH))
            for rep in range(reps):
                pp = rep % 2
                if mode == "mm_loop" and state is not None:
                    emit_s3_loads(c, state, rep, range(2 * NCH))
                    emit_s3_half(c, state, rep, 0)
                    emit_s3_half(c, state, rep, 1)
                    continue
                wq_cur = wq_next
                prev = state
                if prev is not None and mode is None:
                    # h1 g4-7 via HWDGE int8 loads on the scalar ring (off
                    # the gpsimd rail); converted on ACT mid-iteration
                    emit_s3_loads_hw(c, prev, rep, range(NCH + 4, 2 * NCH),
                                     nc.scalar)
                xf_tiles = emit_s1_x(c, rep)
                s1 = emit_s1_main(c, rep, pp, wq_cur, xf_tiles)
                if prev is not None and mode is None:
                    emit_s3_half(c, prev, rep, 0)
                    emit_s3_conv(c, prev, rep, range(NCH + 4, 2 * NCH))
                state = emit_s2(c, rep, pp, s1, wq_cur,
                                skip_ag=(mode == "body_noag"))
                if rep + 1 < reps and mode != "mm_loop":
                    wq_next = emit_w_load(c, rep + 1)
                if prev is not None and mode is None:
                    emit_s3_half(c, prev, rep, 1)
                # ternarize p1b/p2/p3 AFTER all readers of the previous
                # rep's bt ring are emitted (WAR tracking)
                emit_tern23(c, state, rep)
                if mode is None:
                    # remaining lhsT casting loads at the END of the rail:
                    # they execute after this rep's AGs and spill JIT into
                    # the next iteration, keeping AG1(r+1) unblocked
                    emit_s3_loads(c, state, rep + 1, range(NCH + 4))
            if mode is None and state is not None:
                emit_s3_loads(c, state, reps, range(NCH + 4, 2 * NCH))
                emit_s3_half(c, state, reps, 0)
                emit_s3_half(c, state, reps, 1)

    _split_waits(nc)
    return nc


def emit_w_load(c, rep):
    nc = c.nc
    wq = []
    for q in range(4):
        t = c.wq_p.tile([P, WQN, N_LOC], F32, tag="wq", name=f"wq_{rep}_{q}")
        eng = nc.sync if q % 2 == 0 else nc.scalar
        eng.dma_start(
            t[:],
            c.w_in[q * WQN * P : (q + 1) * WQN * P, :].rearrange(
                "(kt p) n -> p kt n", p=P
            ),
        )
        wq.append(t)
    return wq


def emit_s1_x(c, rep):
    nc = c.nc
    xf_tiles = []
    for mt in range(MT_LOC):
        xf = c.xz_p.tile([P, K], F32, tag="xf", name=f"xf_{rep}_{mt}")
        nc.scalar.dma_start(xf[:], c.x_in[mt * P : (mt + 1) * P, :])
        xf_tiles.append(xf)
    return xf_tiles


def emit_s1_main(c, rep, pp, wq, xf_tiles, act_hook=None):
    """Stats (DVE) + wsum (ACT) + partition reduce + AllGather #1 + stp."""
    nc = c.nc

    # |W| sums on ACT first (W was prefetched last iter; ready at t=0)
    ws = c.st_p.tile([P, 2 * NCH], F32, tag="ws", name=f"ws_{rep}")
    for cc in range(2 * NCH):
        q, o = cc // 4, cc % 4
        hc = CH // 2
        nc.scalar.activation(
            out=c.wscr[:], in_=wq[q][:, o * hc : (o + 1) * hc, :],
            func=AF.Abs, bias=0.0, scale=1.0,
            accum_out=ws[:, cc : cc + 1],
        )
    if act_hook is not None:
        act_hook()  # h0 g4-7 lhsT converts on ACT, right after wsum

    # per m-tile: moments -> r, x*rms, per-row absmax (DVE; sqrt on ACT)
    amax_mt = c.st_p.tile([P, MT_LOC], F32, tag="amx", name=f"amx_{rep}")
    r_tiles = []
    for mt in range(MT_LOC):
        xf = xf_tiles[mt]
        xg = xf[:].rearrange("p (g d) -> p g d", d=512)
        stats6 = c.st_p.tile([P, 8, 6], F32, tag="st6", name=f"st6_{rep}_{mt}")
        for g in range(8):
            nc.vector.bn_stats(out=stats6[:, g, :], in_=xg[:, g, :])
        mv = c.st_p.tile([P, 2], F32, tag="mv", name=f"mv_{rep}_{mt}")
        nc.vector.bn_aggr(out=mv, in_=stats6[:])
        msq = c.st_p.tile([P, 1], F32, tag=f"msq{mt}", name=f"msq_{rep}_{mt}")
        nc.vector.tensor_tensor(out=msq, in0=mv[:, 0:1], in1=mv[:, 0:1],
                                op=ALU.mult)
        nc.vector.tensor_tensor(out=msq, in0=msq, in1=mv[:, 1:2], op=ALU.add)
        r_t = c.st_p.tile([P, 1], F32, tag=f"rt{mt}", name=f"rt_{rep}_{mt}")
        nc.scalar.activation(out=r_t, in_=msq, func=AF.Sqrt,
                             bias=c.eps_t, scale=1.0)
        nc.vector.reciprocal(out=r_t, in_=r_t)
        r_tiles.append(r_t)
        nc.vector.tensor_tensor(out=xf[:], in0=xf[:], in1=c.rms_b[:],
                                op=ALU.mult)
        amax_raw = c.st_p.tile([P, 1], F32, tag=f"amr{mt}",
                               name=f"amr_{rep}_{mt}")
        nc.vector.tensor_reduce(
            out=amax_raw, in_=xf[:], axis=AX.X, op=ALU.max,
            apply_absolute_value=True,
        )
        nc.vector.tensor_tensor(
            out=amax_mt[:, mt : mt + 1], in0=amax_raw, in1=r_t, op=ALU.mult
        )

    pr = c.st_p.tile([P, 2], F32, tag="pr", name=f"pr_{rep}")
    nc.vector.tensor_reduce(out=pr[:, 0:1], in_=amax_mt[:], axis=AX.X, op=ALU.max)
    nc.vector.tensor_reduce(out=pr[:, 1:2], in_=ws[:], axis=AX.X, op=ALU.add)
    nc.sync.dma_start(c.wsc_d[pp][:].rearrange("(p t) -> p t", p=P), pr[:])
    wscb = c.st_p.tile([P, P, 2], F32, tag="wscb", name=f"wscb_{rep}")
    nc.sync.dma_start(
        wscb[:],
        bass.AP(tensor=c.wsc_d[pp][:].tensor, offset=0, ap=[[0, P], [2, P], [1, 2]]),
    )
    pc = c.st_p.tile([P, 2], F32, tag="pc", name=f"pc_{rep}")
    nc.vector.tensor_reduce(
        out=pc[:, 0:1], in_=wscb[:, :, 0:1], axis=AX.XY, op=ALU.max
    )
    nc.vector.tensor_reduce(
        out=pc[:, 1:2], in_=wscb[:, :, 1:2], axis=AX.XY, op=ALU.add
    )
    nc.sync.dma_start(c.sb_loc[pp][:].rearrange("(p t) -> p t", p=P), pc[:])
    nc.gpsimd.collective_compute(
        "AllGather", ALU.bypass, replica_groups=c.rg,
        ins=[c.sb_loc[pp][:]], outs=[c.sb_all[pp][:]],
    )
    stp = c.st_p.tile([P, R, 2], F32, tag="stp", name=f"stp_{rep}")
    nc.sync.dma_start(
        stp[:],
        bass.AP(tensor=c.sb_all[pp][:].tensor, offset=0,
                ap=[[2, P], [P * 2, R], [1, 2]]),
    )

    s1 = _Ctx()
    s1.xf_tiles = xf_tiles
    s1.r_tiles = r_tiles
    s1.stp = stp
    return s1


def emit_s2(c, rep, pp, s1, wq, skip_ag=False):
    """Scales, quantize->transpose->casting int8 stage, AGs, ternarize."""
    nc = c.nc

    stp = s1.stp
    gmax = c.st_p.tile([P, 1], F32, tag="gmax", name=f"gmax_{rep}")
    nc.vector.tensor_reduce(out=gmax, in_=stp[:, :, 0:1], axis=AX.XY, op=ALU.max)
    nc.vector.tensor_scalar_max(out=gmax, in0=gmax, scalar1=Q_CLIP)
    a_s = c.st_p.tile([P, 1], F32, tag="as", name=f"as_{rep}")
    nc.vector.reciprocal(out=a_s, in_=gmax)
    nc.vector.tensor_scalar_mul(out=a_s, in0=a_s, scalar1=127.0)
    gsum = c.st_p.tile([P, 1], F32, tag="gsum", name=f"gsum_{rep}")
    nc.vector.tensor_reduce(out=gsum, in_=stp[:, :, 1:2], axis=AX.XY, op=ALU.add)
    nc.vector.tensor_scalar(
        out=gsum, in0=gsum, scalar1=1.0 / (K * N), scalar2=Q_CLIP,
        op0=ALU.mult, op1=ALU.max,
    )
    b_s = c.st_p.tile([P, 1], F32, tag="bs", name=f"bs_{rep}")
    nc.vector.reciprocal(out=b_s, in_=gsum)
    dq = c.st_p.tile([P, 1], F32, tag="dq", name=f"dq_{rep}")
    nc.vector.tensor_tensor(out=dq, in0=gmax, in1=gsum, op=ALU.mult)
    nc.vector.tensor_scalar_mul(out=dq, in0=dq, scalar1=1.0 / 127.0)

    # ternarize pass 1a (wave A: chunks 0-3) on ACT, bit-exact single-round:
    # wq <- w*b_s + MAGIC in f32 (the add IS the RNE-to-integer)
    def tern_p1a(cc):
        q, o = cc // 2, cc % 2
        sl = wq[q][:, o * CH : (o + 1) * CH, :]
        nc.scalar.activation(
            out=sl, in_=sl, func=AF.Copy, bias=MAGIC, scale=b_s[:, 0:1],
        )

    for cc in range(NCH // 2):
        tern_p1a(cc)

    # quantize + transpose + casting int8 stage, per (m-tile, k-half)
    for mt in range(MT_LOC):
        xf = s1.xf_tiles[mt]
        rs = c.st_p.tile([P, 1], F32, tag=f"rs{mt}", name=f"rs_{rep}_{mt}")
        nc.vector.tensor_tensor(out=rs, in0=s1.r_tiles[mt], in1=a_s, op=ALU.mult)
        nc.vector.tensor_scalar(
            out=xf[:], in0=xf[:], scalar1=rs, scalar2=MAGIC,
            op0=ALU.mult, op1=ALU.add,
        )
        for hk in range(2):
            zbt = c.zb_p.tile([P, KH], BF16, tag="zb", name=f"zb_{rep}_{mt}{hk}")
            nc.vector.tensor_scalar(
                out=zbt[:], in0=xf[:, hk * KH : (hk + 1) * KH], scalar1=MAGIC,
                scalar2=None, op0=ALU.subtract,
            )
            ztp = c.ztp_p.tile([P, HT, P], BF16, tag="ztp",
                               name=f"ztp_{rep}_{mt}{hk}")
            nc.sync.dma_start_transpose(ztp[:], zbt[:])
            # int8 convert on DVE + HWDGE stage (keeps the gpsimd rail free)
            zt8 = c.zt8_p.tile([P, HT, P], I8, tag="zt8",
                               name=f"zt8_{rep}_{mt}{hk}")
            nc.vector.tensor_copy(zt8[:], ztp[:])
            nc.sync.dma_start(
                c.z8_loc[pp][mt * ZT : (mt + 1) * ZT].rearrange(
                    "(p f) -> p f", p=P
                )[:, hk * (HT * P) : (hk + 1) * (HT * P)],
                zt8[:].rearrange("p a b -> p (a b)"),
            )
    if not skip_ag:
        # single AllGather for both m-tiles: one rendezvous per rep
        nc.gpsimd.collective_compute(
            "AllGather", ALU.bypass, replica_groups=c.rg,
            ins=[c.z8_loc[pp][:]], outs=[c.z8_all[pp][:]],
        )

    st = _Ctx()
    st.bt_chunks = [None] * NCH
    st.tern_p1a = tern_p1a
    st.wq = wq
    st.dq = dq
    st.pp = pp
    st.lhsb = {}
    st.lhs8 = {}
    return st


def emit_tern23(c, st, rep):
    """Ternarize pass 1b (DVE: tw = bf16(wq - MAGIC), exact ints) and the
    fused clip into the fp8 bt chunk ring (plus wave-B pass 1a on ACT).
    Emitted only after every reader of the previous rep's bt ring, so the
    pool's WAR deps are tracked in program order."""
    nc = c.nc

    def p23(cc):
        q, o = cc // 2, cc % 2
        sl = st.wq[q][:, o * CH : (o + 1) * CH, :]
        tb = c.tw_p.tile([P, CH, N_LOC], BF16, tag="tw", name=f"tw_{rep}_{cc}")
        nc.vector.tensor_scalar(
            out=tb[:], in0=sl, scalar1=MAGIC, scalar2=None, op0=ALU.subtract
        )
        btc = c.bt_p.tile([P, CH, N_LOC], FP8, tag="bt", name=f"bt_{rep}_{cc}")
        nc.vector.tensor_scalar(
            out=btc[:], in0=tb[:], scalar1=1.0, scalar2=-1.0,
            op0=ALU.min, op1=ALU.max,
        )
        st.bt_chunks[cc] = btc

    for cc in range(NCH // 2):
        p23(cc)
    for cc in range(NCH // 2, NCH):
        st.tern_p1a(cc)
    for cc in range(NCH // 2, NCH):
        p23(cc)


def emit_s3_loads(c, st, rep, idxs):
    """lhsT casting loads (SWDGE: int8 DRAM -> bf16 SBUF), idx = h*NCH+g."""
    nc = c.nc
    pp = st.pp
    for idx in idxs:
        h, g = idx // NCH, idx % NCH
        t = c.lhsb_p.tile([P, R, CH * P], BF16, tag="lhsb",
                          name=f"lb_{rep}_{h}{g}")
        nc.gpsimd.dma_start(
            t[:],
            bass.AP(
                tensor=c.z8_all[pp][:].tensor,
                offset=h * ZT + g * CH * P,
                ap=[[KT * P, P], [MT_LOC * ZT, R], [1, CH * P]],
            ),
        )
        st.lhsb[(h, g)] = t


def emit_s3_loads_hw(c, st, rep, idxs, eng):
    """lhsT loads via HWDGE rings into int8 tiles; converted to bf16 later
    on DVE/ACT (emit_s3_conv*)."""
    nc = c.nc
    pp = st.pp
    for idx in idxs:
        h, g = idx // NCH, idx % NCH
        t = c.lhs8_p.tile([P, R, CH * P], I8, tag="lhs8",
                          name=f"l8_{rep}_{h}{g}")
        eng.dma_start(
            t[:],
            bass.AP(
                tensor=c.z8_all[pp][:].tensor,
                offset=h * ZT + g * CH * P,
                ap=[[KT * P, P], [MT_LOC * ZT, R], [1, CH * P]],
            ),
        )
        st.lhs8[(h, g)] = t


def emit_s3_conv(c, st, rep, idxs):
    """int8 -> bf16 lhsT converts on ACT."""
    nc = c.nc
    for idx in idxs:
        h, g = idx // NCH, idx % NCH
        t = c.lhsb_p.tile([P, R, CH * P], BF16, tag="lhsb",
                          name=f"lc_{rep}_{h}{g}")
        nc.scalar.copy(t[:], st.lhs8[(h, g)][:])
        st.lhsb[(h, g)] = t


def emit_s3_conv_dve(c, st, rep, idxs):
    """int8 -> bf16 lhsT converts on DVE (2x rate for <=16-bit)."""
    nc = c.nc
    for idx in idxs:
        h, g = idx // NCH, idx % NCH
        t = c.lhsb_p.tile([P, R, CH * P], BF16, tag="lhsb",
                          name=f"lc_{rep}_{h}{g}")
        nc.vector.tensor_copy(t[:], st.lhs8[(h, g)][:])
        st.lhsb[(h, g)] = t


def emit_s3_half(c, st, rep, h):
    """One matmul half (m-tile group h): 8 groups x (4 kt x 8 ranks) into 8
    PSUM banks; ACT drains with dequant scale; scalar-queue out stores."""
    nc = c.nc
    lhsb = st.lhsb

    psums = [
        c.psum_p.tile([P, N_LOC], F32, tag="ps", name=f"ps_{rep}_{h}_{i}")
        for i in range(R)
    ]
    for g in range(NCH):
        tb = lhsb[(h, g)]
        btc = st.bt_chunks[g]
        for kk in range(CH):
            for rr in range(R):
                nc.tensor.matmul(
                    psums[rr][:],
                    tb[:, rr, kk * P : (kk + 1) * P],
                    btc[:, kk, :],
                    start=(g == 0 and kk == 0),
                    stop=(g == NCH - 1 and kk == CH - 1),
                )
    for rr in range(R):
        o_t = c.out_p.tile([P, N_LOC], F32, tag="ot",
                           name=f"ot_{rep}_{h}_{rr}")
        nc.scalar.activation(
            out=o_t[:], in_=psums[rr][:], func=AF.Copy,
            bias=0.0, scale=st.dq[:, 0:1],
        )
        gm = 2 * rr + h
        nc.scalar.dma_start(c.out_ext[gm * P : (gm + 1) * P, :], o_t[:])


_CACHE = {}


def _get_nc():
    if "nc" not in _CACHE:
        _CACHE["nc"] = build_kernel()
    return _CACHE["nc"]


def make_in_maps(x, weight, rms_weight):
    x = np.ascontiguousarray(np.asarray(x, dtype=np.float32)).reshape(M, K)
    weight = np.asarray(weight, dtype=np.float32)
    rms_weight = np.ascontiguousarray(np.asarray(rms_weight, dtype=np.float32))
    return [
        {
            "x_loc": np.ascontiguousarray(x[c * M_LOC : (c + 1) * M_LOC]),
            "w_loc": np.ascontiguousarray(weight[:, c * N_LOC : (c + 1) * N_LOC]),
            "rms_w": rms_weight,
        }
        for c in range(R)
    ]


def assemble_out(results):
    out = np.concatenate([results[c]["out_loc"] for c in range(R)], axis=1)
    return out.reshape(1, M, N)


def kernel(x, weight, rms_weight):
    nc = _get_nc()
    in_maps = make_in_maps(x, weight, rms_weight)
    res = run_bass_kernel_spmd(nc, in_maps, core_ids=list(range(R)))
    return assemble_out(res.results)
